# revision 12
# baseline (speedup 1.0000x reference)
"""DGGraphConv (GCN-style message passing) on 8 Trainium2 NeuronCores.

out = segment_sum(edge_weight * x[edge_src], edge_dst) @ W + bias

Reordering: aggregate raw x rows first, GEMM last.  No supp = x @ W
materialization (100 MB) and no collectives: every core receives the full
x (replicated, cast to fp16 host-side), gathers only the rows feeding its
destination-node shard with the custom SWDGE dma_gather, segment-sums them
via scatter-matmuls, and finishes with a small f32r GEMM.

dma_gather uses int16 row indices, so x is addressed in 4 chunks of 25000
rows; edges are grouped per (128-node dst block, src chunk) and padded to
128-edge tiles (pad edges have weight 0).

Per-core device pipeline (identical SPMD program, data differs per core):
  for each super-block of SB_N dst blocks:
    - 4 dma_gather calls (one per src chunk) pull all of the super-block's
      edge source rows (fp16, 512B each) into SBUF
    for each block in the super-block:
      - per 128-edge tile: build S[e,n] = ew[e] * (dst_local[e]==n) with one
        fused DVE tensor_scalar (fp16), matmul-accumulate S.T @ G into PSUM
      - copy PSUM->SBUF, PE-transpose the [128,256] accumulator, 2 f32r
        matmuls with W, add bias, DMA the 128 output rows out
"""

import os

import numpy as np

import concourse.bass as bass
import concourse.mybir as mybir
import concourse.tile as tile
from concourse import bacc, bass_utils

N_NODES = 100000
N_EDGES = 800000
FEAT = 256
N_CORES = 8
P = 128
ROWS_PER_CORE = N_NODES // N_CORES          # 12500
N_BLOCKS = -(-ROWS_PER_CORE // P)           # 98
ROWS_PADDED = N_BLOCKS * P                  # 12544
NCHUNK = 4
CHUNK_BOUNDS = [0, 32768, 65536, 98304, 100000]   # int16-addressable chunks
SB_N = 5                                    # blocks per super-block
N_SB = -(-N_BLOCKS // SB_N)                 # 20

F32 = mybir.dt.float32
F32R = mybir.dt.float32r
F16 = mybir.dt.float16
I16 = mybir.dt.int16

GATHER_DT = {"f16": F16, "bf16": mybir.dt.bfloat16, "f32r": F32R}[
    os.environ.get("KERNEL_GDT", "f16")]
GATHER_NP = {F16: np.float16, mybir.dt.bfloat16: None, F32R: np.float32}[GATHER_DT]
EPI_DT = F32R   # final GEMM dtype


def _build_edge_plan(edge_src, edge_dst, edge_weight):
    """Partition + pad edges per (core, dst block, src chunk).

    Returns (m, per_core):
      m: [N_BLOCKS, NCHUNK] tiles per cell, shared across cores.
      per_core[c] = (idx16 [P, 8*T] int16, win [P, T] f32, ew [P, T] f32)
    Global tile order: for sb, for k, for b in sb, for t in range(m[b,k]).
    """
    core_of = edge_dst // ROWS_PER_CORE
    per_core_raw = []
    cnt = np.zeros((N_CORES, N_BLOCKS, NCHUNK), dtype=np.int64)
    for c in range(N_CORES):
        sel = core_of == c
        src_c = edge_src[sel].astype(np.int64)
        ew_c = edge_weight[sel]
        dl = edge_dst[sel].astype(np.int64) - c * ROWS_PER_CORE
        blk = dl // P
        win = (dl % P).astype(np.float32)
        ch = np.searchsorted(CHUNK_BOUNDS, src_c, side="right") - 1
        key = blk * NCHUNK + ch
        order = np.argsort(key, kind="stable")
        per_core_raw.append((src_c[order], ew_c[order], key[order], win[order]))
        np.add.at(cnt, (c, blk, ch), 1)

    m = -(-cnt.max(axis=0) // P)                       # [NB, NCHUNK]
    empty = m.sum(axis=1) == 0
    m[empty, 0] = 1                                    # ≥1 tile per block
    T = int(m.sum())

    # slot start (in tiles) of cell (b, k) in the global gather order
    tile_off = np.zeros((N_BLOCKS, NCHUNK), dtype=np.int64)
    tt = 0
    for sb in range(N_SB):
        blocks = range(sb * SB_N, min((sb + 1) * SB_N, N_BLOCKS))
        for pos in range(NCHUNK):
            k = (pos + sb) % NCHUNK
            for b in blocks:
                tile_off[b, k] = tt
                tt += m[b, k]
    assert tt == T

    per_core = []
    for c in range(N_CORES):
        src_c, ew_c, key, win = per_core_raw[c]
        # rank within each (b, k) group
        kcnt = cnt[c].reshape(-1)
        cum = np.concatenate([[0], np.cumsum(kcnt)[:-1]])
        rank = np.arange(len(key)) - cum[key]
        pos = tile_off.reshape(-1)[key] * P + rank
        srcl_pad = np.zeros(T * P, dtype=np.int16)
        win_pad = np.zeros(T * P, dtype=np.float32)
        ew_pad = np.zeros(T * P, dtype=np.float32)
        srcl_pad[pos] = (src_c - np.asarray(CHUNK_BOUNDS)[
            np.searchsorted(CHUNK_BOUNDS, src_c, side="right") - 1]).astype(np.int16)
        win_pad[pos] = win
        ew_pad[pos] = ew_c

        # idx16 layout: tile t's 128 idxs occupy columns [8t, 8t+8);
        # idx j -> [16r + (j%16), 8t + j//16] replicated for r in 0..7
        blk16 = srcl_pad.reshape(T, 8, 16)             # [T, col, lane]
        idx16 = np.zeros((P, 8 * T), dtype=np.int16)
        lanes = blk16.transpose(2, 0, 1).reshape(16, 8 * T)
        for r in range(8):
            idx16[16 * r:16 * (r + 1)] = lanes
        per_core.append((
            np.ascontiguousarray(idx16),
            np.ascontiguousarray(win_pad.reshape(T, P).T),
            np.ascontiguousarray(ew_pad.reshape(T, P).T),
        ))
    return m, per_core


def _build_nc(m):
    m = np.asarray(m)
    T = int(m.sum())
    # dynamic_dma_scratch_size funds the SWDGE descriptor rings: with 4
    # queues the per-queue ring is scratch/16/4 descs per engine.  16 KiB
    # (default) holds only ~2 gather calls per queue, so the ucode's
    # await_space serializes gather gen behind the previous call's DMA
    # completion — 32 KiB gives ~4-call depth and lets gathers pipeline
    # ahead of compute.
    nc = bacc.Bacc("TRN2", target_bir_lowering=False, debug=False,
                   num_swdge_queues=4, dynamic_dma_scratch_size=32768)

    x16 = nc.dram_tensor("x16", [N_NODES, FEAT], GATHER_DT,
                         kind="ExternalInput").ap()
    w = nc.dram_tensor("w", [2 * P, FEAT], EPI_DT, kind="ExternalInput").ap()
    bias_bc = nc.dram_tensor("bias_bc", [P, FEAT], F32, kind="ExternalInput").ap()
    iota = nc.dram_tensor("iota", [P, P], GATHER_DT, kind="ExternalInput").ap()
    ident = nc.dram_tensor("ident", [P, P], F32, kind="ExternalInput").ap()
    idx16 = nc.dram_tensor("idx16", [P, 8 * T], I16, kind="ExternalInput").ap()
    dst_win = nc.dram_tensor("dst_win", [P, T], F32, kind="ExternalInput").ap()
    ew_in = nc.dram_tensor("ew", [P, T], F32, kind="ExternalInput").ap()
    out = nc.dram_tensor("out", [ROWS_PADDED, FEAT], F32, kind="ExternalOutput").ap()

    # per-(sb, k) gather call sizes, and max per k for pool sizing
    call_tiles = np.zeros((N_SB, NCHUNK), dtype=np.int64)
    for sb in range(N_SB):
        blocks = range(sb * SB_N, min((sb + 1) * SB_N, N_BLOCKS))
        for k in range(NCHUNK):
            call_tiles[sb, k] = sum(int(m[b, k]) for b in blocks)
    gmax = int(call_tiles.max())

    with tile.TileContext(nc) as tc:
        with (
            tc.tile_pool(name="consts", bufs=1) as cpool,
            tc.tile_pool(name="gpool", bufs=3) as gpool,
            tc.tile_pool(name="spool", bufs=6) as spool,
            tc.tile_pool(name="accsb", bufs=2) as accsb_pool,
            tc.tile_pool(name="outsb", bufs=3) as outsb_pool,
            tc.tile_pool(name="psacc", bufs=2, space="PSUM") as ps_acc,
            tc.tile_pool(name="pstp", bufs=2, space="PSUM") as ps_tp,
            tc.tile_pool(name="psout", bufs=2, space="PSUM") as ps_out,
        ):
            w_sb = cpool.tile([P, 2 * FEAT], EPI_DT)
            nc.sync.dma_start(out=w_sb[:, 0:FEAT], in_=w[0:P, :])
            nc.sync.dma_start(out=w_sb[:, FEAT:2 * FEAT], in_=w[P:2 * P, :])
            bias_sb = cpool.tile([P, FEAT], F32)
            nc.sync.dma_start(out=bias_sb[:], in_=bias_bc[:])
            iota_sb = cpool.tile([P, P], GATHER_DT)
            nc.sync.dma_start(out=iota_sb[:], in_=iota[:])
            ident_sb = cpool.tile([P, P], F32)
            nc.sync.dma_start(out=ident_sb[:], in_=ident[:])
            idx_sb = cpool.tile([P, 8 * T], I16)
            nc.sync.dma_start(out=idx_sb[:], in_=idx16[:])
            dst_sb = cpool.tile([P, T], F32)
            nc.sync.dma_start(out=dst_sb[:], in_=dst_win[:])
            ew_sb = cpool.tile([P, T], F32)
            nc.sync.dma_start(out=ew_sb[:], in_=ew_in[:])

            tt = 0          # global tile counter (gather order)
            for sb in range(N_SB):
                blocks = list(range(sb * SB_N, min((sb + 1) * SB_N, N_BLOCKS)))
                # gather: one call per chunk
                g_k = [None] * NCHUNK
                base_k = [0] * NCHUNK
                for pos in range(NCHUNK):
                    k = (pos + sb) % NCHUNK
                    n = int(call_tiles[sb, k])
                    base_k[k] = tt
                    g = gpool.tile([P, max(n, 1) * FEAT], GATHER_DT,
                                   tag=f"g{k}", padded_shape=[P, gmax * FEAT],
                                   name=f"g{k}")
                    g_k[k] = g
                    assert n > 0, (sb, k)
                    g3 = g[:].rearrange("p (c f) -> p c f", f=FEAT)
                    nc.gpsimd.dma_gather(
                        out_ap=g3,
                        in_ap=x16[CHUNK_BOUNDS[k]:CHUNK_BOUNDS[k + 1], :],
                        idxs_ap=idx_sb[:, 8 * tt:8 * (tt + n)],
                        num_idxs=n * P,
                        num_idxs_reg=n * P,
                        elem_size=FEAT,
                        single_packet=False,
                        queue_num=(1, 2, 3, 0)[pos],
                    )
                    tt += n

                # compute per block
                for b in blocks:
                    n_tiles_b = int(m[b].sum())
                    acc = ps_acc.tile([P, FEAT], F32, tag="acc")
                    done = 0
                    for k in range(NCHUNK):
                        # position of b's tiles within call (sb, k)
                        pos = sum(int(m[b2, k]) for b2 in blocks if b2 < b)
                        gcol = base_k[k] + pos
                        for t in range(int(m[b, k])):
                            s = spool.tile([P, P], GATHER_DT, tag="s")
                            nc.vector.tensor_scalar(
                                out=s[:],
                                in0=iota_sb[:],
                                scalar1=dst_sb[:, gcol + t:gcol + t + 1],
                                scalar2=ew_sb[:, gcol + t:gcol + t + 1],
                                op0=mybir.AluOpType.is_equal,
                                op1=mybir.AluOpType.mult,
                            )
                            goff = (pos + t) * FEAT
                            nc.tensor.matmul(
                                out=acc[:],
                                lhsT=s[:],
                                rhs=g_k[k][:, goff:goff + FEAT],
                                start=(done == 0),
                                stop=(done == n_tiles_b - 1),
                            )
                            done += 1
                    assert done == n_tiles_b

                    acc_sb = accsb_pool.tile([P, FEAT], F32, tag="acc_sb")
                    nc.scalar.copy(out=acc_sb[:], in_=acc[:])
                    accT_sb = accsb_pool.tile([P, FEAT], EPI_DT, tag="accT_sb")
                    for h in range(2):
                        tp = ps_tp.tile([P, P], F32, tag="tp")
                        nc.tensor.transpose(
                            out=tp[:], in_=acc_sb[:, h * P:(h + 1) * P],
                            identity=ident_sb[:])
                        nc.scalar.copy(
                            out=accT_sb[:, h * P:(h + 1) * P], in_=tp[:])
                    outp = ps_out.tile([P, FEAT], F32, tag="outp")
                    for h in range(2):
                        nc.tensor.matmul(
                            out=outp[:],
                            lhsT=accT_sb[:, h * P:(h + 1) * P],
                            rhs=w_sb[:, h * FEAT:(h + 1) * FEAT],
                            start=(h == 0),
                            stop=(h == 1),
                        )
                    out_t = outsb_pool.tile([P, FEAT], F32, tag="out_t")
                    nc.vector.tensor_tensor(
                        out=out_t[:], in0=outp[:], in1=bias_sb[:],
                        op=mybir.AluOpType.add)
                    nc.sync.dma_start(out=out[b * P:(b + 1) * P, :], in_=out_t[:])
            assert tt == T

    nc.compile()
    return nc


def _install_ntff_hook():
    """Register the axon NTFF profile hook that this image's antenv lacks."""
    import sys
    import types

    try:
        from antenv.axon_hooks import get_axon_ntff_profile_hook  # noqa: F401
        return True
    except ImportError:
        pass
    try:
        import antenv
        from trn_agent_boot.trn_boot import _ntff_profile_via_ctypes
    except ImportError:
        return False
    hook = _ntff_profile_via_ctypes("/opt/axon/libaxon_pjrt.so")
    if hook is None:
        return False
    mod = types.ModuleType("antenv.axon_hooks")
    mod._hook = hook
    mod.set_axon_ntff_profile_hook = lambda h: setattr(mod, "_hook", h)
    mod.get_axon_ntff_profile_hook = lambda: mod._hook
    sys.modules["antenv.axon_hooks"] = mod
    antenv.axon_hooks = mod
    return True


_NC_CACHE = {}
LAST_EXEC_TIME_NS = None


def _get_nc(m):
    key = m.tobytes()
    if key not in _NC_CACHE:
        _NC_CACHE[key] = _build_nc(m)
    return _NC_CACHE[key]


def kernel(x, weight, bias, edge_weight, edge_src, edge_dst):
    x = np.ascontiguousarray(np.asarray(x, dtype=np.float32))
    weight = np.ascontiguousarray(np.asarray(weight, dtype=np.float32))
    bias = np.asarray(bias, dtype=np.float32)
    edge_weight = np.asarray(edge_weight, dtype=np.float32)
    edge_src = np.asarray(edge_src, dtype=np.int32)
    edge_dst = np.asarray(edge_dst, dtype=np.int32)

    m, per_core = _build_edge_plan(edge_src, edge_dst, edge_weight)
    nc = _get_nc(m)

    if GATHER_DT == F16:
        x_g = x.astype(np.float16)
    elif GATHER_DT == mybir.dt.bfloat16:
        import ml_dtypes
        x_g = x.astype(ml_dtypes.bfloat16)
    else:
        x_g = x
    bias_bc = np.ascontiguousarray(np.broadcast_to(bias.reshape(1, FEAT), (P, FEAT)))
    iota = np.ascontiguousarray(np.broadcast_to(
        np.arange(P, dtype=np.float32).reshape(1, P), (P, P))).astype(GATHER_NP)
    ident = np.eye(P, dtype=np.float32)

    in_maps = []
    for c in range(N_CORES):
        idx16_c, win_c, ew_c = per_core[c]
        in_maps.append({
            "x16": x_g,
            "w": weight,
            "bias_bc": bias_bc,
            "iota": iota,
            "ident": ident,
            "idx16": idx16_c,
            "dst_win": win_c,
            "ew": ew_c,
        })

    global LAST_EXEC_TIME_NS
    trace = os.environ.get("KERNEL_TRACE", "0") == "1"
    kw = {}
    if trace:
        if _install_ntff_hook():
            bass_utils.upload_artifacts = lambda tmpdir: tmpdir
            kw = dict(trace=True, trace_cores=list(range(N_CORES)))
        else:
            print("KERNEL_TRACE requested but NTFF hook unavailable")
    res = bass_utils.run_bass_kernel_spmd(
        nc, in_maps, core_ids=list(range(N_CORES)), **kw)
    if trace:
        LAST_EXEC_TIME_NS = res.exec_time_ns
        print(f"KERNEL_EXEC_TIME_NS: {res.exec_time_ns}")
        print(f"KERNEL_MEAN_EXEC_TIME_NS: {res.mean_exec_time_ns}")
        if res.instructions_and_trace is not None:
            print(f"KERNEL_TRACE_PATH: {res.instructions_and_trace[1]}")

    out = np.empty((N_NODES, FEAT), dtype=np.float32)
    for c in range(N_CORES):
        out[c * ROWS_PER_CORE:(c + 1) * ROWS_PER_CORE] = \
            res.results[c]["out"][:ROWS_PER_CORE]
    return out

